# revision 1
# baseline (speedup 1.0000x reference)
"""Trainium2 Bass kernel for LocationSensitiveAttention.

Strategy (data-parallel over batch, 8 cores x 16 batches):
  per batch b:
    V   [128, 8, 512]  <- memory[b] natural layout (t on partitions, f32r)
    VT  [128, 4, 1024] <- PE-transposed V (d on partitions) via 32 128x128 transposes
    comb[128(a), 512(t)] psum x2 = W_memory^T @ V^T  +  M2plus^T @ XTplus
        where M2plus = [conv_kernel@W_loc ; pq_b] (pq folded via ones row in XT)
    tanh -> [128(a), 1024(t)] sbuf (ACT)
    energies [128(t), 8(tc)] psum via 8 matmuls with v as rhs
    softmax over 1024 = (partitions x 8 cols), no max-sub (|e| <= ||v||_1 ~ 9)
    context [1, 512] psum = sum_t attn[t] * V[t, :]  (8 accumulating matmuls)
All heavy matmul operands are float32r (fp32 RNE-rounded to 11 mantissa bits,
pre-rounded on host), which streams at 1 cyc/row on the PE (4x faster than fp32).
"""
import sys

sys.path.insert(0, "/opt/trn_rl_repo")
import numpy as np

import concourse.bacc as bacc
import concourse.bass as bass
import concourse.mybir as mybir
import concourse.tile as tile
from concourse.bass_utils import run_bass_kernel_spmd

F32 = mybir.dt.float32
F32R = mybir.dt.float32r
AP = bass.AP

NCORES = 8
B, T, D, Q, A, NF, KW = 128, 1024, 512, 1024, 128, 32, 31
BL = B // NCORES          # 16 batches per core
TC = T // 128             # 8 t-chunks
DC = D // 128             # 4 d-chunks
KC = 2 * KW               # 62 conv taps (c-major: rows 0..30 prev, 31..61 cum)
PAD = (KW - 1) // 2       # 15
TP = T + 2 * PAD          # 1054
NEG = -1.0e38

LAST_EXEC_NS = None


def f32r_round(x):
    """Host equivalent of the device fp32->fp32r cast (RNE to 11 mantissa bits)."""
    u = np.ascontiguousarray(x, dtype=np.float32).view(np.uint32).astype(np.uint64)
    tie = (u >> np.uint64(12)) & np.uint64(1)
    r = (((u + np.uint64(0x7FF) + tie) >> np.uint64(12)) << np.uint64(12)).astype(np.uint32)
    return r.view(np.float32)


def build_program(n_batches=BL):
    nc = bacc.Bacc(None, target_bir_lowering=False)
    d = {}

    def inp(name, shape, dt):
        d[name] = nc.dram_tensor(name, list(shape), dt, kind="ExternalInput")
        return d[name]

    def outp(name, shape, dt):
        d[name] = nc.dram_tensor(name, list(shape), dt, kind="ExternalOutput")
        return d[name]

    mem_d = inp("mem", (n_batches, T, D), F32R)
    ppad_d = inp("ppad", (n_batches, TP), F32R)
    cpad_d = inp("cpad", (n_batches, TP), F32R)
    pcum_d = inp("pcum", (n_batches, T), F32)
    qt_d = inp("qt", (Q, n_batches), F32R)
    wq_d = inp("wq", (Q, A), F32R)
    wm_d = inp("wm", (D, A), F32R)
    m2_d = inp("m2", (KC, n_batches * A), F32R)
    v_d = inp("v", (A, 1), F32)
    ident_d = inp("ident", (128, 128), F32R)
    onesrow_d = inp("onesrow", (1, T), F32R)
    ones128_d = inp("ones128", (128, 1), F32)
    onesm_d = inp("onesm", (1, 128), F32)
    iota_d = inp("iota", (128, TC), F32)
    seql_d = inp("seql", (128, n_batches), F32)

    ctx_d = outp("ctx_o", (n_batches, D), F32)
    attn_d = outp("attn_o", (n_batches, T), F32)
    ncum_d = outp("ncum_o", (n_batches, T), F32)

    Add = mybir.AluOpType.add
    Tanh = mybir.ActivationFunctionType.Tanh
    Exp = mybir.ActivationFunctionType.Exp

    with tile.TileContext(nc) as tc:
        with (
            tc.tile_pool(name="static", bufs=1) as st,
            tc.tile_pool(name="vpool", bufs=3) as vpool,
            tc.tile_pool(name="vtpool", bufs=2) as vtpool,
            tc.tile_pool(name="xtpool", bufs=2) as xtpool,
            tc.tile_pool(name="thpool", bufs=2) as thpool,
            tc.tile_pool(name="small", bufs=2) as sm,
            tc.tile_pool(name="vtps", bufs=2, space=bass.MemorySpace.PSUM) as vtps_p,
            tc.tile_pool(name="comb", bufs=2, space=bass.MemorySpace.PSUM) as comb_p,
            tc.tile_pool(name="ep", bufs=4, space=bass.MemorySpace.PSUM) as ep_p,
        ):
            # ---- statics ----
            wm = st.tile([128, DC, A], F32R, tag="wm")
            nc.sync.dma_start(wm[:], AP(wm_d, 0, [[A, 128], [128 * A, DC], [1, A]]))
            wq = st.tile([128, TC, A], F32R, tag="wq")
            nc.sync.dma_start(wq[:], AP(wq_d, 0, [[A, 128], [128 * A, TC], [1, A]]))
            qt = st.tile([128, TC, n_batches], F32R, tag="qt")
            nc.sync.dma_start(
                qt[:], AP(qt_d, 0, [[n_batches, 128], [128 * n_batches, TC], [1, n_batches]])
            )
            m2p = st.tile([KC + 1, n_batches * A], F32R, tag="m2p")
            nc.sync.dma_start(m2p[0:KC, :], m2_d[:])
            ident = st.tile([128, 128], F32R, tag="ident")
            nc.sync.dma_start(ident[:], ident_d[:])
            vv = st.tile([A, 1], F32, tag="vv")
            nc.sync.dma_start(vv[:], v_d[:])
            ones128 = st.tile([128, 1], F32, tag="ones128")
            nc.sync.dma_start(ones128[:], ones128_d[:])
            onesm = st.tile([1, 128], F32, tag="onesm")
            nc.sync.dma_start(onesm[:], onesm_d[:])
            iota = st.tile([128, TC], F32, tag="iota")
            nc.sync.dma_start(iota[:], iota_d[:])
            seql = st.tile([128, n_batches], F32, tag="seql")
            nc.sync.dma_start(seql[:], seql_d[:])

            # ---- pq = query @ W_query, scattered into row 62 of m2p ----
            pq_ps = ep_p.tile([n_batches, A], F32, tag="ep")
            for c in range(TC):
                nc.tensor.matmul(
                    pq_ps[:], qt[:, c, :], wq[:, c, :], start=(c == 0), stop=(c == TC - 1)
                )
            pq_sb = sm.tile([n_batches, A], F32R, tag="pq_sb")
            nc.vector.tensor_copy(pq_sb[:], pq_ps[:])
            nc.sync.dma_start(m2p[KC : KC + 1, :], pq_sb[:])

            for b in range(n_batches):
                # ---- load memory[b] natural: [128(t%128), 8(tc), 512(d)] ----
                V = vpool.tile([128, TC, D], F32R, tag="V")
                nc.sync.dma_start(
                    V[:], AP(mem_d, b * T * D, [[D, 128], [128 * D, TC], [1, D]])
                )
                # ---- im2col windows for the location conv (+ ones row) ----
                XT = xtpool.tile([KC + 1, T], F32R, tag="XT")
                nc.sync.dma_start(XT[0:KW, :], AP(ppad_d, b * TP, [[1, KW], [1, T]]))
                nc.sync.dma_start(XT[KW:KC, :], AP(cpad_d, b * TP, [[1, KW], [1, T]]))
                nc.sync.dma_start(XT[KC : KC + 1, :], onesrow_d[:])

                # ---- transpose V -> VT [128(d%128), 4(dc), 1024(t)] ----
                VT = vtpool.tile([128, DC, T], F32R, tag="VT")
                for h in range(2):
                    for dc in range(DC):
                        ps = vtps_p.tile([128, 512], F32R, tag="vtps")
                        for i in range(4):
                            tcc = h * 4 + i
                            nc.tensor.matmul(
                                ps[:, i * 128 : (i + 1) * 128],
                                V[:, tcc, dc * 128 : (dc + 1) * 128],
                                ident[:],
                                is_transpose=True,
                                start=(i == 0),
                                stop=(i == 3),
                            )
                        eng = nc.vector if (dc % 2 == 0) else nc.scalar
                        if eng is nc.vector:
                            nc.vector.tensor_copy(
                                VT[:, dc, h * 512 : (h + 1) * 512], ps[:]
                            )
                        else:
                            nc.scalar.copy(VT[:, dc, h * 512 : (h + 1) * 512], ps[:])

                # ---- comb = keys^T + ploc^T + pq  (PSUM [128(a), 512(t)] x2) ----
                th = thpool.tile([128, T], F32, tag="th")
                for h in range(2):
                    cps = comb_p.tile([128, 512], F32, tag="comb")
                    for dc in range(DC):
                        nc.tensor.matmul(
                            cps[:],
                            wm[:, dc, :],
                            VT[:, dc, h * 512 : (h + 1) * 512],
                            start=(dc == 0),
                            stop=False,
                        )
                    nc.tensor.matmul(
                        cps[:],
                        m2p[:, b * A : (b + 1) * A],
                        XT[:, h * 512 : (h + 1) * 512],
                        start=False,
                        stop=True,
                    )
                    nc.scalar.activation(th[:, h * 512 : (h + 1) * 512], cps[:], Tanh)

                # ---- energies [128(t), 8(tc)] ----
                eps = ep_p.tile([128, TC], F32, tag="ep")
                for tcc in range(TC):
                    nc.tensor.matmul(
                        eps[:, tcc : tcc + 1],
                        th[:, tcc * 128 : (tcc + 1) * 128],
                        vv[:],
                        start=(tcc == 0),
                        stop=(tcc == TC - 1),
                    )

                # ---- masked softmax over (partition,col) without max-sub ----
                pen = sm.tile([128, TC], F32, tag="pen")
                nc.vector.tensor_scalar(
                    pen[:], iota[:], seql[:, b : b + 1], NEG,
                    mybir.AluOpType.is_ge, mybir.AluOpType.mult,
                )
                em = sm.tile([128, TC], F32, tag="em")
                nc.vector.tensor_tensor(em[:], eps[:], pen[:], Add)
                expm = sm.tile([128, TC], F32, tag="expm")
                scol = sm.tile([128, 1], F32, tag="scol")
                nc.scalar.activation(expm[:], em[:], Exp, accum_out=scol[:])
                s_ps = ep_p.tile([1, 1], F32, tag="ep")
                nc.tensor.matmul(s_ps[:], ones128[:], scol[:], start=True, stop=True)
                r_sb = sm.tile([1, 1], F32, tag="r_sb")
                nc.vector.reciprocal(r_sb[:], s_ps[:])
                rb_ps = ep_p.tile([128, 1], F32, tag="ep")
                nc.tensor.matmul(rb_ps[:], onesm[:], r_sb[:], start=True, stop=True)
                r128 = sm.tile([128, 1], F32, tag="r128")
                nc.scalar.copy(r128[:], rb_ps[:])
                attn = sm.tile([128, TC], F32, tag="attn")
                nc.vector.tensor_scalar_mul(attn[:], expm[:], r128[:])
                attn_r = sm.tile([128, TC], F32R, tag="attn_r")
                nc.vector.tensor_copy(attn_r[:], attn[:])

                # ---- cumulative weights ----
                pct = sm.tile([128, TC], F32, tag="pct")
                nc.sync.dma_start(pct[:], AP(pcum_d, b * T, [[1, 128], [128, TC]]))
                ncum = sm.tile([128, TC], F32, tag="ncum")
                nc.vector.tensor_tensor(ncum[:], attn[:], pct[:], Add)
                nc.sync.dma_start(AP(attn_d, b * T, [[1, 128], [128, TC]]), attn[:])
                nc.sync.dma_start(AP(ncum_d, b * T, [[1, 128], [128, TC]]), ncum[:])

                # ---- context [1, 512] ----
                cxp = ep_p.tile([1, D], F32, tag="ep")
                for tcc in range(TC):
                    nc.tensor.matmul(
                        cxp[:],
                        attn_r[:, tcc : tcc + 1],
                        V[:, tcc, :],
                        start=(tcc == 0),
                        stop=(tcc == TC - 1),
                    )
                cxr = sm.tile([1, D], F32, tag="cxr")
                nc.scalar.copy(cxr[:], cxp[:])
                nc.sync.dma_start(ctx_d[b : b + 1, :], cxr[:])

    nc.compile()
    return nc


def make_in_maps(query, prev_attn_weights, prev_attn_weights_cum, memory,
                 memory_sequence_length, W_query, W_memory, conv_kernel, W_loc, v,
                 n_batches=BL, n_cores=NCORES):
    memory_r = f32r_round(memory)
    query_r = f32r_round(query)
    prev_r = f32r_round(prev_attn_weights)
    cum_r = f32r_round(prev_attn_weights_cum)
    wq_r = np.ascontiguousarray(f32r_round(W_query))
    wm_r = np.ascontiguousarray(f32r_round(W_memory))
    # fold conv kernel into W_loc: M2[(c*31+k), a] = sum_f ck[k,c,f] W_loc[f,a]
    m2 = np.einsum(
        "kcf,fa->cka",
        conv_kernel.astype(np.float64),
        W_loc.astype(np.float64),
    ).reshape(KC, A).astype(np.float32)
    m2_rep = np.ascontiguousarray(np.tile(f32r_round(m2), (1, n_batches)))
    iota = (np.arange(TC, dtype=np.float32)[None, :] * 128
            + np.arange(128, dtype=np.float32)[:, None])
    shared = {
        "wq": wq_r,
        "wm": wm_r,
        "m2": m2_rep,
        "v": np.ascontiguousarray(v.astype(np.float32)),
        "ident": np.eye(128, dtype=np.float32),
        "onesrow": np.ones((1, T), dtype=np.float32),
        "ones128": np.ones((128, 1), dtype=np.float32),
        "onesm": np.ones((1, 128), dtype=np.float32),
        "iota": np.ascontiguousarray(iota),
    }
    in_maps = []
    for c in range(n_cores):
        b0 = c * n_batches
        sl = slice(b0, b0 + n_batches)
        ppad = np.zeros((n_batches, TP), dtype=np.float32)
        ppad[:, PAD : PAD + T] = prev_r[sl]
        cpad = np.zeros((n_batches, TP), dtype=np.float32)
        cpad[:, PAD : PAD + T] = cum_r[sl]
        seql = np.broadcast_to(
            memory_sequence_length[sl].astype(np.float32)[None, :], (128, n_batches)
        )
        in_maps.append(dict(
            shared,
            mem=np.ascontiguousarray(memory_r[sl]),
            ppad=ppad,
            cpad=cpad,
            pcum=np.ascontiguousarray(prev_attn_weights_cum[sl].astype(np.float32)),
            qt=np.ascontiguousarray(query_r[sl].T),
            seql=np.ascontiguousarray(seql),
        ))
    return in_maps


_NC_CACHE = {}


def kernel(query, prev_attn_weights, prev_attn_weights_cum, memory,
           memory_sequence_length, W_query, W_memory, conv_kernel, W_loc, v):
    global LAST_EXEC_NS
    query = np.asarray(query, dtype=np.float32)
    prev_attn_weights = np.asarray(prev_attn_weights, dtype=np.float32)
    prev_attn_weights_cum = np.asarray(prev_attn_weights_cum, dtype=np.float32)
    memory = np.asarray(memory, dtype=np.float32)
    memory_sequence_length = np.asarray(memory_sequence_length)
    W_query = np.asarray(W_query, dtype=np.float32)
    W_memory = np.asarray(W_memory, dtype=np.float32)
    conv_kernel = np.asarray(conv_kernel, dtype=np.float32)
    W_loc = np.asarray(W_loc, dtype=np.float32)
    v = np.asarray(v, dtype=np.float32)

    if "nc" not in _NC_CACHE:
        _NC_CACHE["nc"] = build_program(BL)
    nc = _NC_CACHE["nc"]

    in_maps = make_in_maps(
        query, prev_attn_weights, prev_attn_weights_cum, memory,
        memory_sequence_length, W_query, W_memory, conv_kernel, W_loc, v,
    )
    import os
    trace = bool(os.environ.get("KERNEL_TRACE"))
    out = run_bass_kernel_spmd(nc, in_maps, list(range(NCORES)), trace=trace)
    LAST_EXEC_NS = out.exec_time_ns
    res = out.results
    context = np.concatenate([res[c]["ctx_o"] for c in range(NCORES)], axis=0)
    attn = np.concatenate([res[c]["attn_o"] for c in range(NCORES)], axis=0)
    ncum = np.concatenate([res[c]["ncum_o"] for c in range(NCORES)], axis=0)
    return context, attn, ncum


# revision 3
# speedup vs baseline: 1.3577x; 1.3577x over previous
"""Trainium2 Bass kernel for LocationSensitiveAttention.

Strategy (data-parallel over batch, 8 cores x 16 batches):
  per batch b:
    V   [128, 8, 512]  <- memory[b] natural layout (t on partitions, f32r)
    VT  [128, 4, 1024] <- PE-transposed V (d on partitions) via 32 128x128 transposes
    comb[128(a), 512(t)] psum x2 = W_memory^T @ V^T  +  M2plus^T @ XTplus
        where M2plus = [conv_kernel@W_loc ; pq_b] (pq folded via ones row in XT)
    tanh -> [128(a), 1024(t)] sbuf (ACT)
    energies [128(t), 8(tc)] psum via 8 matmuls with v as rhs
    softmax over 1024 = (partitions x 8 cols), no max-sub (|e| <= ||v||_1 ~ 9)
    context [1, 512] psum = sum_t attn[t] * V[t, :]  (8 accumulating matmuls)
All heavy matmul operands are float32r (fp32 RNE-rounded to 11 mantissa bits,
pre-rounded on host), which streams at 1 cyc/row on the PE (4x faster than fp32).
"""
import sys

sys.path.insert(0, "/opt/trn_rl_repo")
import numpy as np

import concourse.bacc as bacc
import concourse.bass as bass
import concourse.mybir as mybir
import concourse.tile as tile
from concourse.bass_utils import run_bass_kernel_spmd

F32 = mybir.dt.float32
F32R = mybir.dt.float32r
AP = bass.AP

NCORES = 8
B, T, D, Q, A, NF, KW = 128, 1024, 512, 1024, 128, 32, 31
BL = B // NCORES          # 16 batches per core
TC = T // 128             # 8 t-chunks
DC = D // 128             # 4 d-chunks
KC = 2 * KW               # 62 conv taps (c-major: rows 0..30 prev, 31..61 cum)
PAD = (KW - 1) // 2       # 15
TP = T + 2 * PAD          # 1054
NEG = -1.0e38

LAST_EXEC_NS = None


def f32r_round(x):
    """Host equivalent of the device fp32->fp32r cast (RNE to 11 mantissa bits)."""
    u = np.ascontiguousarray(x, dtype=np.float32).view(np.uint32).astype(np.uint64)
    tie = (u >> np.uint64(12)) & np.uint64(1)
    r = (((u + np.uint64(0x7FF) + tie) >> np.uint64(12)) << np.uint64(12)).astype(np.uint32)
    return r.view(np.float32)


def build_program(n_batches=BL):
    nc = bacc.Bacc(None, target_bir_lowering=False)
    d = {}

    def inp(name, shape, dt):
        d[name] = nc.dram_tensor(name, list(shape), dt, kind="ExternalInput")
        return d[name]

    def outp(name, shape, dt):
        d[name] = nc.dram_tensor(name, list(shape), dt, kind="ExternalOutput")
        return d[name]

    mem_d = inp("mem", (n_batches, T, D), F32R)
    ppad_d = inp("ppad", (n_batches, TP), F32R)
    cpad_d = inp("cpad", (n_batches, TP), F32R)
    pcum_d = inp("pcum", (n_batches, T), F32)
    qt_d = inp("qt", (Q, n_batches), F32R)
    wq_d = inp("wq", (Q, A), F32R)
    wm_d = inp("wm", (D, A), F32R)
    m2_d = inp("m2", (KC, n_batches * A), F32R)
    v_d = inp("v", (A, 1), F32)
    ident_d = inp("ident", (128, 128), F32R)
    onesrow_d = inp("onesrow", (1, T), F32R)
    ones128_d = inp("ones128", (128, 1), F32)
    onesm_d = inp("onesm", (1, 128), F32)
    iota_d = inp("iota", (128, TC), F32)
    seql_d = inp("seql", (128, n_batches), F32)

    ctx_d = outp("ctx_o", (n_batches, D), F32)
    attn_d = outp("attn_o", (n_batches, T), F32)
    ncum_d = outp("ncum_o", (n_batches, T), F32)

    Add = mybir.AluOpType.add
    Tanh = mybir.ActivationFunctionType.Tanh
    Exp = mybir.ActivationFunctionType.Exp

    with tile.TileContext(nc) as tc:
        with (
            tc.tile_pool(name="static", bufs=1) as st,
            tc.tile_pool(name="vpool", bufs=4) as vpool,
            tc.tile_pool(name="vtpool", bufs=2) as vtpool,
            tc.tile_pool(name="xtpool", bufs=2) as xtpool,
            tc.tile_pool(name="thpool", bufs=2) as thpool,
            tc.tile_pool(name="small", bufs=2) as sm,
            tc.tile_pool(name="vtps", bufs=2, space=bass.MemorySpace.PSUM) as vtps_p,
            tc.tile_pool(name="comb", bufs=2, space=bass.MemorySpace.PSUM) as comb_p,
            tc.tile_pool(name="ep", bufs=4, space=bass.MemorySpace.PSUM) as ep_p,
        ):
            # ---- statics ----
            wm = st.tile([128, DC, A], F32R, tag="wm")
            nc.sync.dma_start(wm[:], AP(wm_d, 0, [[A, 128], [128 * A, DC], [1, A]]))
            wq = st.tile([128, TC, A], F32R, tag="wq")
            nc.sync.dma_start(wq[:], AP(wq_d, 0, [[A, 128], [128 * A, TC], [1, A]]))
            qt = st.tile([128, TC, n_batches], F32R, tag="qt")
            nc.sync.dma_start(
                qt[:], AP(qt_d, 0, [[n_batches, 128], [128 * n_batches, TC], [1, n_batches]])
            )
            m2p = st.tile([KC + 1, n_batches * A], F32R, tag="m2p")
            nc.sync.dma_start(m2p[0:KC, :], m2_d[:])
            ident = st.tile([128, 128], F32R, tag="ident")
            nc.sync.dma_start(ident[:], ident_d[:])
            vv = st.tile([A, 1], F32, tag="vv")
            nc.sync.dma_start(vv[:], v_d[:])
            ones128 = st.tile([128, 1], F32, tag="ones128")
            nc.sync.dma_start(ones128[:], ones128_d[:])
            onesm = st.tile([1, 128], F32, tag="onesm")
            nc.sync.dma_start(onesm[:], onesm_d[:])
            iota = st.tile([128, TC], F32, tag="iota")
            nc.sync.dma_start(iota[:], iota_d[:])
            seql = st.tile([128, n_batches], F32, tag="seql")
            nc.sync.dma_start(seql[:], seql_d[:])

            # ---- pq = query @ W_query, scattered into row 62 of m2p ----
            pq_ps = ep_p.tile([n_batches, A], F32, tag="ep")
            for c in range(TC):
                nc.tensor.matmul(
                    pq_ps[:], qt[:, c, :], wq[:, c, :], start=(c == 0), stop=(c == TC - 1)
                )
            pq_sb = sm.tile([n_batches, A], F32R, tag="pq_sb")
            nc.vector.tensor_copy(pq_sb[:], pq_ps[:])
            nc.sync.dma_start(m2p[KC : KC + 1, :], pq_sb[:])

            for b in range(n_batches):
                # ---- load memory[b] natural: [128(t%128), 8(tc), 512(d)] ----
                V = vpool.tile([128, TC, D], F32R, tag="V")
                nc.sync.dma_start(
                    V[:], AP(mem_d, b * T * D, [[D, 128], [128 * D, TC], [1, D]])
                )
                # ---- im2col windows for the location conv (+ ones row) ----
                XT = xtpool.tile([KC + 1, T], F32R, tag="XT")
                nc.sync.dma_start(XT[0:KW, :], AP(ppad_d, b * TP, [[1, KW], [1, T]]))
                nc.sync.dma_start(XT[KW:KC, :], AP(cpad_d, b * TP, [[1, KW], [1, T]]))
                nc.sync.dma_start(XT[KC : KC + 1, :], onesrow_d[:])

                # ---- transpose V -> VT [128(d%128), 4(dc), 1024(t)] ----
                VT = vtpool.tile([128, DC, T], F32R, tag="VT")
                for h in range(2):
                    for dc in range(DC):
                        ps = vtps_p.tile([128, 512], F32R, tag="vtps")
                        for i in range(4):
                            tcc = h * 4 + i
                            nc.tensor.matmul(
                                ps[:, i * 128 : (i + 1) * 128],
                                V[:, tcc, dc * 128 : (dc + 1) * 128],
                                ident[:],
                                is_transpose=True,
                                start=(i == 0),
                                stop=(i == 3),
                            )
                        eng = nc.vector if (dc % 2 == 0) else nc.scalar
                        if eng is nc.vector:
                            nc.vector.tensor_copy(
                                VT[:, dc, h * 512 : (h + 1) * 512], ps[:]
                            )
                        else:
                            nc.scalar.copy(VT[:, dc, h * 512 : (h + 1) * 512], ps[:])

                # ---- comb = keys^T + ploc^T + pq  (PSUM [128(a), 512(t)] x2) ----
                th = thpool.tile([128, T], F32, tag="th")
                for h in range(2):
                    cps = comb_p.tile([128, 512], F32, tag="comb")
                    for dc in range(DC):
                        nc.tensor.matmul(
                            cps[:],
                            wm[:, dc, :],
                            VT[:, dc, h * 512 : (h + 1) * 512],
                            start=(dc == 0),
                            stop=False,
                        )
                    nc.tensor.matmul(
                        cps[:],
                        m2p[:, b * A : (b + 1) * A],
                        XT[:, h * 512 : (h + 1) * 512],
                        start=False,
                        stop=True,
                    )
                    nc.scalar.activation(th[:, h * 512 : (h + 1) * 512], cps[:], Tanh)

                # ---- energies [128(t), 8(tc)] ----
                eps = ep_p.tile([128, TC], F32, tag="ep")
                for tcc in range(TC):
                    nc.tensor.matmul(
                        eps[:, tcc : tcc + 1],
                        th[:, tcc * 128 : (tcc + 1) * 128],
                        vv[:],
                        start=(tcc == 0),
                        stop=(tcc == TC - 1),
                    )

                # ---- masked softmax over (partition,col) without max-sub ----
                pen = sm.tile([128, TC], F32, tag="pen")
                nc.vector.tensor_scalar(
                    pen[:], iota[:], seql[:, b : b + 1], NEG,
                    mybir.AluOpType.is_ge, mybir.AluOpType.mult,
                )
                em = sm.tile([128, TC], F32, tag="em")
                nc.vector.tensor_tensor(em[:], eps[:], pen[:], Add)
                expm = sm.tile([128, TC], F32, tag="expm")
                scol = sm.tile([128, 1], F32, tag="scol")
                nc.scalar.activation(expm[:], em[:], Exp, accum_out=scol[:])
                s_ps = ep_p.tile([1, 1], F32, tag="ep")
                nc.tensor.matmul(s_ps[:], ones128[:], scol[:], start=True, stop=True)
                r_sb = sm.tile([1, 1], F32, tag="r_sb")
                nc.vector.reciprocal(r_sb[:], s_ps[:])
                rb_ps = ep_p.tile([128, 1], F32, tag="ep")
                nc.tensor.matmul(rb_ps[:], onesm[:], r_sb[:], start=True, stop=True)
                r128 = sm.tile([128, 1], F32, tag="r128")
                nc.scalar.copy(r128[:], rb_ps[:])
                attn = sm.tile([128, TC], F32, tag="attn")
                nc.vector.tensor_scalar_mul(attn[:], expm[:], r128[:])
                attn_r = sm.tile([128, TC], F32R, tag="attn_r")
                nc.vector.tensor_copy(attn_r[:], attn[:])

                # ---- transpose attn [128,8] -> rows [8,128] for efficient IO ----
                atp = ep_p.tile([TC, 128], F32, tag="ep")
                nc.tensor.matmul(
                    atp[:], attn[:], ident[:].bitcast(F32),
                    is_transpose=True, start=True, stop=True,
                )
                attn_row = sm.tile([TC, 128], F32, tag="attn_row")
                nc.scalar.copy(attn_row[:], atp[:])
                nc.sync.dma_start(AP(attn_d, b * T, [[128, TC], [1, 128]]), attn_row[:])
                # ---- cumulative weights in row layout ----
                pcr = sm.tile([TC, 128], F32, tag="pcr")
                nc.sync.dma_start(pcr[:], AP(pcum_d, b * T, [[128, TC], [1, 128]]))
                ncum_row = sm.tile([TC, 128], F32, tag="ncum_row")
                nc.vector.tensor_tensor(ncum_row[:], attn_row[:], pcr[:], Add)
                nc.sync.dma_start(AP(ncum_d, b * T, [[128, TC], [1, 128]]), ncum_row[:])

                # ---- context [1, 512] ----
                cxp = ep_p.tile([1, D], F32, tag="ep")
                for tcc in range(TC):
                    nc.tensor.matmul(
                        cxp[:],
                        attn_r[:, tcc : tcc + 1],
                        V[:, tcc, :],
                        start=(tcc == 0),
                        stop=(tcc == TC - 1),
                    )
                cxr = sm.tile([1, D], F32, tag="cxr")
                nc.scalar.copy(cxr[:], cxp[:])
                nc.sync.dma_start(ctx_d[b : b + 1, :], cxr[:])

    nc.compile()
    return nc


def make_in_maps(query, prev_attn_weights, prev_attn_weights_cum, memory,
                 memory_sequence_length, W_query, W_memory, conv_kernel, W_loc, v,
                 n_batches=BL, n_cores=NCORES):
    memory_r = f32r_round(memory)
    query_r = f32r_round(query)
    prev_r = f32r_round(prev_attn_weights)
    cum_r = f32r_round(prev_attn_weights_cum)
    wq_r = np.ascontiguousarray(f32r_round(W_query))
    wm_r = np.ascontiguousarray(f32r_round(W_memory))
    # fold conv kernel into W_loc: M2[(c*31+k), a] = sum_f ck[k,c,f] W_loc[f,a]
    m2 = np.einsum(
        "kcf,fa->cka",
        conv_kernel.astype(np.float64),
        W_loc.astype(np.float64),
    ).reshape(KC, A).astype(np.float32)
    m2_rep = np.ascontiguousarray(np.tile(f32r_round(m2), (1, n_batches)))
    iota = (np.arange(TC, dtype=np.float32)[None, :] * 128
            + np.arange(128, dtype=np.float32)[:, None])
    shared = {
        "wq": wq_r,
        "wm": wm_r,
        "m2": m2_rep,
        "v": np.ascontiguousarray(v.astype(np.float32)),
        "ident": np.eye(128, dtype=np.float32),
        "onesrow": np.ones((1, T), dtype=np.float32),
        "ones128": np.ones((128, 1), dtype=np.float32),
        "onesm": np.ones((1, 128), dtype=np.float32),
        "iota": np.ascontiguousarray(iota),
    }
    in_maps = []
    for c in range(n_cores):
        b0 = c * n_batches
        sl = slice(b0, b0 + n_batches)
        ppad = np.zeros((n_batches, TP), dtype=np.float32)
        ppad[:, PAD : PAD + T] = prev_r[sl]
        cpad = np.zeros((n_batches, TP), dtype=np.float32)
        cpad[:, PAD : PAD + T] = cum_r[sl]
        seql = np.broadcast_to(
            memory_sequence_length[sl].astype(np.float32)[None, :], (128, n_batches)
        )
        in_maps.append(dict(
            shared,
            mem=np.ascontiguousarray(memory_r[sl]),
            ppad=ppad,
            cpad=cpad,
            pcum=np.ascontiguousarray(prev_attn_weights_cum[sl].astype(np.float32)),
            qt=np.ascontiguousarray(query_r[sl].T),
            seql=np.ascontiguousarray(seql),
        ))
    return in_maps


_NC_CACHE = {}


def kernel(query, prev_attn_weights, prev_attn_weights_cum, memory,
           memory_sequence_length, W_query, W_memory, conv_kernel, W_loc, v):
    global LAST_EXEC_NS
    query = np.asarray(query, dtype=np.float32)
    prev_attn_weights = np.asarray(prev_attn_weights, dtype=np.float32)
    prev_attn_weights_cum = np.asarray(prev_attn_weights_cum, dtype=np.float32)
    memory = np.asarray(memory, dtype=np.float32)
    memory_sequence_length = np.asarray(memory_sequence_length)
    W_query = np.asarray(W_query, dtype=np.float32)
    W_memory = np.asarray(W_memory, dtype=np.float32)
    conv_kernel = np.asarray(conv_kernel, dtype=np.float32)
    W_loc = np.asarray(W_loc, dtype=np.float32)
    v = np.asarray(v, dtype=np.float32)

    if "nc" not in _NC_CACHE:
        _NC_CACHE["nc"] = build_program(BL)
    nc = _NC_CACHE["nc"]

    in_maps = make_in_maps(
        query, prev_attn_weights, prev_attn_weights_cum, memory,
        memory_sequence_length, W_query, W_memory, conv_kernel, W_loc, v,
    )
    import os
    trace = bool(os.environ.get("KERNEL_TRACE"))
    out = run_bass_kernel_spmd(nc, in_maps, list(range(NCORES)), trace=trace)
    LAST_EXEC_NS = out.exec_time_ns
    res = out.results
    context = np.concatenate([res[c]["ctx_o"] for c in range(NCORES)], axis=0)
    attn = np.concatenate([res[c]["attn_o"] for c in range(NCORES)], axis=0)
    ncum = np.concatenate([res[c]["ncum_o"] for c in range(NCORES)], axis=0)
    return context, attn, ncum
